# revision 19
# baseline (speedup 1.0000x reference)
"""MixtureLinearAttention TRN2 kernel (8 NeuronCores, SPMD).

Math (per batch n, component c, head h):
  Qf = elu(q @ W_c) + 1 ;  Kf = (elu(k @ W_c) + 1) * mask
  KVt[e, m] = sum_s Kf[s,e] V[s,m] ;  Ksum[e] = sum_s Kf[s,e]
  Den[s] = sum_e Qf[s,e] Ksum[e]
  out[s,h,m] = sum_c softmax(mix)_c / Den[s] * sum_e Qf[s,e] KVt[e,m]

Sharding: core i -> (n = i//2, heads hh = (i%2)*8..+8). Host does all layout
transposes. All matmul operands fp16 (fp32 HIGH-mode matmuls are 4x slower);
PSUM accumulation stays fp32 so only input-rounding error is added.

Per h-pair pipeline (software-pipelined: the z-scaled output phase of h-pair
N is emitted during h-pair N+1's compute so its broadcast DMAs hide):
  phi-K:  lhsT = kT chunk, rhs = blockdiag(Wcat)  -> proj psum [128s, 512]
          ACT Exp -> ek fp16 ; POOL ts min(ek,1) -> akk (offloads DVE)
          DVE stt (proj max 0) add akk -> kf fp16
  KV:     lhsT = Vaug(s,65) fp16, rhs = kf slices -> [65, 512] psum f32
  kvc:    DVE ts (kv * 1/256) -> fp16  (256-scale keeps fp16 zq normal)
  KVt:    4 fp16 is_transpose matmuls [65,128]->[128,65] direct into one
          [128, 260] fp16 psum tile; DVE drains at 2x (fp16 psum read).
  phi-Q:  lhsT = Wcat-pair [64, 128] fp16 (K=64, q NOT duplicated)
          ACT Exp + ACT Relu (psum drains); ONE DVE stt (min 1) add relu
          (all-SBUF fp16 -> 4x DVE mode) -> qf fp16 c-pair stacks [128ce, S]
  Den:    per-stack lhsT [128,8] (ksum/256 * 1/W cols) -> den [8, 512] psum
  1/x:    DVE reciprocal psum -> z fp16 [8, 2048]   (z = 256W/den)
  zbc:    z -> DRAM roundtrip, then per-stack broadcast DMA with stride-0
          source AP (row 2i -> partitions 0:64, 2i+1 -> 64:128). Replaces
          the selector-matmul partition broadcast (frees ~25% of PE time;
          DVE/PE cannot partition-broadcast, gpsimd can but is too slow).
  zq:     DVE tensor_tensor(qf, zb) all-SBUF fp16 (2x mode); OutT: packed
          [128, 512] psum, both heads via tile_position, accumulate over p
          NOTE: psum start_tensor_calc resets per-PARTITION: the first MM
          writing each partition range of a bank needs start=True.
  out:    ACT-Copy / DVE-ts alternate psum -> sbuf fp16, DMA to DRAM
          (DMA cannot read PSUM; gpsimd cannot access PSUM at all)
"""
import sys

if "/opt/trn_rl_repo" not in sys.path:
    sys.path.insert(0, "/opt/trn_rl_repo")

from contextlib import ExitStack

import numpy as np

import concourse.bass as bass
import concourse.tile as tile
from concourse import mybir
from concourse.masks import make_identity

F32 = mybir.dt.float32
F16 = mybir.dt.float16
ALU = mybir.AluOpType
AFT = mybir.ActivationFunctionType

N, S, H, D, C = 4, 2048, 16, 64, 4
E = M = 64
HL = 8          # heads per core
NHP = HL // 2   # h-pairs
NCHUNK = S // 128
OSCALE = 256.0  # kv scaled by 1/256, z by 256 -> fp16 zq stays normal


def _split_multiwait(nc, max_waits=1):
    """This walrus build rejects >1 sync wait per instruction; hoist extra
    waits onto NoOps inserted just before, on the same engine."""
    k = 0
    for fn in nc.m.functions:
        for bb in fn.blocks:
            out, changed = [], False
            for inst in bb.instructions:
                si = inst.sync_info
                if si is not None and si.on_wait and len(si.on_wait) > max_waits:
                    waits = list(si.on_wait)
                    while len(waits) > max_waits:
                        chunk, waits = waits[:max_waits], waits[max_waits:]
                        nop = mybir.InstNoOp(name=f"wait_split_{k}", ins=[], outs=[])
                        k += 1
                        nop.engine = inst.engine
                        nop.sync_info = mybir.SyncInfo(on_wait=chunk, on_update=[])
                        out.append(nop)
                        changed = True
                    inst.sync_info = mybir.SyncInfo(
                        on_wait=waits, on_update=list(si.on_update or [])
                    )
                out.append(inst)
            if changed:
                bb.instructions = out


def build_program():
    nc = bass.Bass("TRN2", debug=False)
    ap = {}
    ap["qT"] = nc.dram_tensor("qT", [HL, 128, S], F16, kind="ExternalInput").ap()
    ap["kT"] = nc.dram_tensor("kT", [NHP, 128, S], F16, kind="ExternalInput").ap()
    ap["vaug"] = nc.dram_tensor("vaug", [HL, 128, NCHUNK * 65], F16, kind="ExternalInput").ap()
    ap["wqc"] = nc.dram_tensor("wqc", [2, 128, 128], F16, kind="ExternalInput").ap()
    ap["wk"] = nc.dram_tensor("wk", [128, 512], F16, kind="ExternalInput").ap()
    ap["wmask"] = nc.dram_tensor("wmask", [128, 6], F32, kind="ExternalInput").ap()
    ap["sel"] = nc.dram_tensor("sel", [8, 512], F16, kind="ExternalInput").ap()
    ap["outT"] = nc.dram_tensor("outT", [HL * 64, S], F16, kind="ExternalOutput").ap()

    tc = tile.TileContext(nc)
    with tc:
        with ExitStack() as ctx, nc.allow_low_precision("fp16 kernel by design"):
            cpool = ctx.enter_context(tc.tile_pool(name="consts", bufs=1))
            wk_t = cpool.tile([128, 512], F16)
            nc.sync.dma_start(wk_t[:], ap["wk"][:])
            wq_t = []
            for p in range(2):
                w1 = cpool.tile([128, 128], F16, name=f"wq{p}", tag=f"wq{p}")
                nc.sync.dma_start(w1[:], ap["wqc"][p])
                wq_t.append(w1)
            wmask_t = cpool.tile([128, 6], F32)
            nc.sync.dma_start(wmask_t[:], ap["wmask"][:])
            ident = cpool.tile([128, 128], F16)
            make_identity(nc, ident[:])
            sel_t = cpool.tile([8, 512], F16, name="sel", tag="sel")
            nc.sync.dma_start(sel_t[:], ap["sel"][:])

            qd_pool = ctx.enter_context(tc.tile_pool(name="qd", bufs=2))
            kt_pool = ctx.enter_context(tc.tile_pool(name="kt", bufs=3))
            v_pool = ctx.enter_context(tc.tile_pool(name="v", bufs=2))
            kf_pool = ctx.enter_context(tc.tile_pool(name="kf", bufs=10))
            etK_pool = ctx.enter_context(tc.tile_pool(name="etK", bufs=4))
            etQ_pool = ctx.enter_context(tc.tile_pool(name="etQ", bufs=4))
            qf_pool = ctx.enter_context(tc.tile_pool(name="qf", bufs=8))
            kvc_pool = ctx.enter_context(tc.tile_pool(name="kvc", bufs=2))
            kstk_pool = ctx.enter_context(tc.tile_pool(name="kstk", bufs=2))
            dl_pool = ctx.enter_context(tc.tile_pool(name="dl", bufs=8))
            z_pool = ctx.enter_context(tc.tile_pool(name="z", bufs=2))
            lnt_pool = ctx.enter_context(tc.tile_pool(name="lnt", bufs=2))
            zd_pool = ctx.enter_context(tc.tile_pool(name="zd", bufs=2, space="DRAM"))
            zb_pool = ctx.enter_context(tc.tile_pool(name="zb", bufs=2))
            zq_pool = ctx.enter_context(tc.tile_pool(name="zq", bufs=4))
            ob_pool = ctx.enter_context(tc.tile_pool(name="ob", bufs=4))
            # PSUM (8 banks): bigK 2 + bigQ 2 + aux(kv|den|tpk) 2 + out 2
            ps_bigK = ctx.enter_context(tc.tile_pool(name="psbigK", bufs=2, space="PSUM"))
            ps_bigQ = ctx.enter_context(tc.tile_pool(name="psbigQ", bufs=1, space="PSUM"))
            ps_aux = ctx.enter_context(tc.tile_pool(name="psaux", bufs=2, space="PSUM"))
            ps_out = ctx.enter_context(tc.tile_pool(name="psout", bufs=2, space="PSUM"))

            prev = None  # (h0, qf2, kstk, zb) of previous h-pair

            def emit_out_ch4(h0p, qf2p, kstkp, zsrc, ch4):
                sl = slice(512 * ch4, 512 * ch4 + 512)
                mode, zval = zsrc
                ot_ps = ps_out.tile([128, 512], F32, name="otps", tag="ot")
                for j in range(2):
                    for p in range(2):
                        i = 2 * j + p
                        zq_t = zq_pool.tile([128, 512], F16, name="zqt", tag="zq")
                        if mode == "zb":
                            nc.vector.tensor_tensor(
                                zq_t[:], qf2p[(j, p)][:, sl],
                                zval[(j, p)][:, sl], ALU.mult,
                            )
                        else:
                            # selector-matmul broadcast (bigK ring idle here)
                            zr_ps = ps_bigK.tile([128, 512], F32, name="zrps", tag="bigK")
                            nc.tensor.matmul(
                                zr_ps[:],
                                sel_t[:, 128 * i : 128 * i + 128],
                                zval[:, sl],
                                start=True,
                                stop=True,
                            )
                            nc.vector.tensor_tensor(
                                zq_t[:], qf2p[(j, p)][:, sl], zr_ps[:], ALU.mult
                            )
                        nc.tensor.matmul(
                            ot_ps[64 * j : 64 * j + 64, :],
                            kstkp[(j, p)][:, 0:64],
                            zq_t[:],
                            start=(p == 0),
                            stop=(p == 1),
                        )
                ob = ob_pool.tile([128, 512], F16, name="ob", tag="ob")
                if ch4 % 4 == 0:
                    nc.scalar.activation(ob[:], ot_ps[:], AFT.Copy)
                else:
                    nc.vector.tensor_scalar(ob[:], ot_ps[:], 1.0, None, ALU.mult)
                nc.gpsimd.dma_start(ap["outT"][h0p * 64 : h0p * 64 + 128, sl], ob[:])


            for hp in range(NHP):
                h0 = 2 * hp
                kt_t = kt_pool.tile([128, S], F16, name="kt")
                if hp == 0:  # split first loads: phi-K starts after 1/4 of kt
                    for q4 in range(4):
                        nc.sync.dma_start(
                            kt_t[:, 512 * q4 : 512 * q4 + 512],
                            ap["kT"][hp][:, 512 * q4 : 512 * q4 + 512],
                        )
                else:
                    nc.sync.dma_start(kt_t[:], ap["kT"][hp])
                v_ts = []
                for j in range(2):
                    v_t = v_pool.tile([128, NCHUNK * 65], F16, name=f"v{j}", tag=f"v{j}")
                    if hp == 0:
                        nc.gpsimd.dma_start(v_t[:, 0:520], ap["vaug"][h0 + j][:, 0:520])
                        nc.gpsimd.dma_start(v_t[:, 520:1040], ap["vaug"][h0 + j][:, 520:1040])
                    else:
                        nc.sync.dma_start(v_t[:], ap["vaug"][h0 + j])
                    v_ts.append(v_t)
                qd_ts = []
                for j in range(2):
                    qd_t = qd_pool.tile([128, S], F16, name=f"qd{j}", tag=f"qd{j}")
                    (nc.gpsimd if hp == 0 else nc.sync).dma_start(
                        qd_t[:], ap["qT"][h0 + j]
                    )
                    qd_ts.append(qd_t)

                # ---- phi-K chunks interleaved 2:1 with phi-Q halves so the
                # DVE-heavy K chain and ACT-heavy Q chain overlap ----
                def phi_q_half(idx):
                    jq, pq, half = idx // 4, (idx // 2) % 2, idx % 2
                    if half == 0:
                        qf2[(jq, pq)] = qf_pool.tile(
                            [128, S], F16, name=f"qf{jq}{pq}", tag="qf"
                        )
                    qf_t = qf2[(jq, pq)]
                    pq_ps = ps_bigQ.tile([128, 1024], F32, name="pqps", tag="bigQ")
                    for g in range(2):
                        o = 1024 * half + 512 * g
                        nc.tensor.matmul(
                            pq_ps[:, 512 * g : 512 * g + 512],
                            wq_t[pq][:],
                            qd_ts[jq][:, o : o + 512],
                            start=True,
                            stop=True,
                        )
                    eq = etQ_pool.tile([128, 1024], F16, name="eq", tag="eq")
                    nc.scalar.activation(eq[:], pq_ps[:], AFT.Exp)
                    xr = etQ_pool.tile([128, 1024], F16, name="xr", tag="xr")
                    nc.scalar.activation(xr[:], pq_ps[:], AFT.Relu)
                    aq = etQ_pool.tile([128, 1024], F16, name="aq", tag="aq")
                    nc.vector.tensor_scalar(aq[:], eq[:], 1.0, None, ALU.min)
                    nc.vector.tensor_tensor(
                        qf_t[:, 1024 * half : 1024 * half + 1024],
                        aq[:], xr[:], ALU.add,
                    )

                qf2 = {}
                kv_ps = ps_aux.tile([65, 512], F32, name="kvps", tag="aux")
                for ch in range(NCHUNK):
                    kf_ps = ps_bigK.tile([128, 512], F32, name="kfps", tag="bigK")
                    nc.tensor.matmul(
                        kf_ps[:],
                        kt_t[:, 128 * ch : 128 * ch + 128],
                        wk_t[:],
                        start=True,
                        stop=True,
                    )
                    ek = etK_pool.tile([128, 512], F16, name="ek", tag="ek")
                    nc.scalar.activation(ek[:], kf_ps[:], AFT.Exp)
                    akk = etK_pool.tile([128, 512], F16, name="akk", tag="akk")
                    nc.vector.tensor_scalar(akk[:], ek[:], 1.0, None, ALU.min)
                    kf = kf_pool.tile([128, 512], F16, name="kf")
                    nc.vector.scalar_tensor_tensor(
                        kf[:], kf_ps[:], 0.0, akk[:], ALU.max, ALU.add
                    )
                    for j in range(2):
                        nc.tensor.matmul(
                            kv_ps[:, 256 * j : 256 * j + 256],
                            v_ts[j][:, 65 * ch : 65 * ch + 65],
                            kf[:, 256 * j : 256 * j + 256],
                            start=(ch == 0 and j == 0),
                            stop=(ch == NCHUNK - 1),
                        )
                    if ch % 2 == 1:
                        phi_q_half(ch // 2)

                # ---- kv -> fp16 with 1/256 fold, transpose into kstk stacks ----
                kvc_t = kvc_pool.tile([65, 512], F16, name="kvc", tag="kvc")
                nc.scalar.activation(kvc_t[:], kv_ps[:], AFT.Copy, scale=1.0 / OSCALE)
                tpk_ps = ps_aux.tile([128, 264], F16, name="tpk", tag="aux")
                for j in range(2):
                    for p in range(2):
                        i = 2 * j + p
                        # NOTE: psum start_tensor_calc resets the full
                        # partition row: only the first write gets start=True.
                        nc.tensor.matmul(
                            tpk_ps[:, 66 * i : 66 * i + 65],
                            kvc_t[:, 256 * j + 128 * p : 256 * j + 128 * p + 128],
                            ident[:65, :65],
                            start=(i == 0),
                            stop=True,
                            is_transpose=True,
                        )
                kst = kstk_pool.tile([128, 264], F16, name="kstk", tag="kstk")
                nc.vector.tensor_scalar(kst[:], tpk_ps[:], 1.0, None, ALU.mult)
                kstk = {}
                for j in range(2):
                    for p in range(2):
                        i = 2 * j + p
                        kstk[(j, p)] = kst[:, 66 * i : 66 * i + 65]

                # ---- dlw: [128, 8] per stack, cols 2i+c = ksum*wmask, rest 0 ----
                dlw = {}
                for j in range(2):
                    for p in range(2):
                        i = 2 * j + p
                        dl = dl_pool.tile([128, 8], F16, name=f"dl{j}{p}", tag=f"dl{j}{p}")
                        nc.gpsimd.memset(dl[:], 0.0)
                        ks = kstk[(j, p)][:, 64:65]
                        nc.gpsimd.tensor_scalar(
                            dl[:, 2 * i : 2 * i + 1], ks,
                            wmask_t[:, 3 * p : 3 * p + 1], None, ALU.mult,
                        )
                        nc.gpsimd.tensor_scalar(
                            dl[:, 2 * i + 1 : 2 * i + 2], ks,
                            wmask_t[:, 3 * p + 1 : 3 * p + 2], None, ALU.mult,
                        )
                        dlw[(j, p)] = dl


                # ---- Den + z per s-group, interleaved with the deferred
                # output phase of the previous h-pair: the den matmuls fill
                # the PE pipe while prev's zq (DVE) ops run, keeping the
                # tensor engine dense (p-state stays high) ----
                z_t = z_pool.tile([8, S], F16, name="zt", tag="z")
                for ch4 in range(4):
                    sl = slice(512 * ch4, 512 * ch4 + 512)
                    den_ps = ps_aux.tile([8, 512], F32, name="denps", tag="aux")
                    for i, (j, p) in enumerate(((0, 0), (0, 1), (1, 0), (1, 1))):
                        nc.tensor.matmul(
                            den_ps[:],
                            dlw[(j, p)][:],
                            qf2[(j, p)][:, sl],
                            start=(i == 0),
                            stop=(i == 3),
                        )
                    lnt = lnt_pool.tile([8, 512], F32, name="lnt", tag="lnt")
                    nc.scalar.activation(lnt[:], den_ps[:], AFT.Ln)
                    nc.scalar.activation(z_t[:, sl], lnt[:], AFT.Exp, scale=-1.0)
                    if prev is not None:
                        emit_out_ch4(*prev, ch4)

                if hp < NHP - 1:
                    # z -> DRAM -> per-stack partition-broadcast DMA (removes
                    # the 16 zrep selector matmuls of this h-pair's out phase)
                    zd = zd_pool.tile([8, S], F16, name="zd", tag="zd")
                    nc.gpsimd.dma_start(zd[:], z_t[:])
                    zb = {}
                    for j in range(2):
                        for p in range(2):
                            i = 2 * j + p
                            zb_t = zb_pool.tile([128, S], F16, name=f"zb{i}", tag=f"zb{i}")
                            nc.gpsimd.dma_start(
                                zb_t[0:64, :],
                                zd[2 * i : 2 * i + 1].broadcast_to([64, S]),
                            )
                            nc.gpsimd.dma_start(
                                zb_t[64:128, :],
                                zd[2 * i + 1 : 2 * i + 2].broadcast_to([64, S]),
                            )
                            zb[(j, p)] = zb_t
                    prev = (h0, qf2, kstk, ("zb", zb))
                else:
                    # final h-pair: PE-local zrep broadcast (no DMA latency in
                    # the tail; the PE is otherwise idle by now)
                    for ch4 in range(4):
                        emit_out_ch4(h0, qf2, kstk, ("zt", z_t), ch4)

    _split_multiwait(nc)
    return nc


_NC_CACHE = None


def _get_nc():
    global _NC_CACHE
    if _NC_CACHE is None:
        _NC_CACHE = build_program()
    return _NC_CACHE


def _softmax(x):
    e = np.exp(x - x.max())
    return e / e.sum()


def prep_core_inputs(queries, keys, values, key_mask, feat_W, mix_weights, core):
    n, hh = core // 2, (core % 2) * HL
    W = _softmax(np.asarray(mix_weights, np.float64)).astype(np.float32)

    qs = queries[n][:, hh : hh + HL, :].transpose(1, 2, 0)
    qT = np.ascontiguousarray(
        np.concatenate([qs, qs], axis=1)
    ).astype(np.float16)  # [HL, 128, S]
    ks = keys[n][:, hh : hh + HL, :].transpose(1, 2, 0)
    kT = np.ascontiguousarray(ks.reshape(NHP, 128, S)).astype(np.float16)

    mask = key_mask[n].astype(np.float32)
    vm = values[n][:, hh : hh + HL, :] * mask[:, None, None]
    vaug = np.concatenate(
        [vm, np.broadcast_to(mask[:, None, None], (S, HL, 1))], axis=2
    )
    vaug = vaug.transpose(1, 0, 2).reshape(HL, NCHUNK, 128, 65)
    vaug = np.ascontiguousarray(vaug.transpose(0, 2, 1, 3)).reshape(
        HL, 128, NCHUNK * 65
    ).astype(np.float16)

    wqc = np.zeros((2, 128, 128), np.float16)
    for p in range(2):
        wqc[p, :64, :64] = feat_W[2 * p]
        wqc[p, 64:, 64:] = feat_W[2 * p + 1]
    wcat = np.concatenate([feat_W[c] for c in range(C)], axis=1)
    wk = np.zeros((128, 512), np.float16)
    wk[:64, :256] = wcat
    wk[64:, 256:] = wcat

    # den lhsT scale: ksum arrives as ksum/256 (kv 1/256 fold); wmask = 1/W
    # makes den_psum = den_true/(256 W) so z = 1/den_psum = 256 W / den.
    wmask = np.zeros((128, 6), np.float32)
    for p in range(2):
        wmask[:64, 3 * p + 0] = 1.0 / W[2 * p]
        wmask[64:, 3 * p + 1] = 1.0 / W[2 * p + 1]

    sel = np.zeros((8, 512), np.float16)
    for i in range(4):
        for c in range(2):
            sel[2 * i + c, 128 * i + 64 * c : 128 * i + 64 * c + 64] = 1.0

    return {"qT": qT, "kT": kT, "vaug": vaug, "wqc": wqc, "wk": wk,
            "wmask": wmask, "sel": sel}


def run_cores(inputs, trace=False, tmpdir=None):
    from concourse.bass_utils import run_bass_kernel_spmd

    nc = _get_nc()
    in_maps = [prep_core_inputs(**inputs, core=i) for i in range(8)]
    kwargs = {}
    if trace:
        kwargs = {"trace": True, "tmpdir": tmpdir}
    res = run_bass_kernel_spmd(nc, in_maps, core_ids=list(range(8)), **kwargs)
    out = np.empty((N, S, H, M), np.float32)
    for i in range(8):
        n, hh = i // 2, (i % 2) * HL
        oT = res.results[i]["outT"].astype(np.float32).reshape(HL, 64, S)
        for h in range(HL):
            out[n, :, hh + h, :] = oT[h].T
    return out, res


def kernel(queries, keys, values, key_mask, feat_W, mix_weights):
    out, _ = run_cores(
        dict(queries=np.asarray(queries), keys=np.asarray(keys),
             values=np.asarray(values), key_mask=np.asarray(key_mask),
             feat_W=np.asarray(feat_W), mix_weights=np.asarray(mix_weights))
    )
    return out
